# revision 1
# baseline (speedup 1.0000x reference)
"""Causal multi-head attention block (B=2, T=2048, C=1024, H=16) on 8 TRN2
NeuronCores.

Sharding: Megatron-style tensor parallel over heads. Core r owns heads
{2r, 2r+1} (output dims [128r, 128r+128) of Wq/Wk/Wv). The final output
projection contracts over all of C, so cores AllGather their local yT
shards (concat on the partition axis == feature axis) into yT_full
[C, B*T], then each core computes a 128-column shard of the output:
outT_shard = Wo[128r:128r+128, :] @ yT_full.

Everything on-device is computed in the "transposed" orientation
(feature-major, token-minor) so the TensorEngine contraction axis always
sits on SBUF partitions and the softmax denominator arrives for free via
a ones-column appended to V:

  qT/kT/vT [128, 4096] = W_shard @ x^T          (x^T passed from host)
  ST tile [128k, 512q] = kT_slice.T @ qT_slice  (contract d=64)
  causal mask: add a -1e9 strictly-lower-triangular matrix into the St
      PSUM accumulation group via matmul(ident, mneg) on diagonal blocks
  PT = exp(ST * 1/sqrt(d))                      (no max-subtraction: logits
                                                 are ~N(0,1), |S|max ~ 6)
  yT [65, 512] += [v | 1].T @ PT                (row 64 = softmax denom)
  yT_norm = yT[0:64] * partition_broadcast(recip(yT[64]))

Performance structure (derived from perfetto/HAM analysis):
- k-tiles are processed in pairs sharing one 2-bank PSUM tile so each EXP
  covers 1024 columns (the ACT engine has a ~352-cycle fixed cost per
  instruction; exp throughput is the attention-phase floor).
- Engine queues execute in order, so anything that may wait on a
  collective is program-ordered well after independent work: the
  AllGather runs in 8 per-(b,jq) chunks fired as soon as their tokens
  finish, and each chunk's O-projection is consumed 3+ chunks later.
- The PE clock throttles (HAM) when the matmul stream has gaps:
  dependency-free warmup matmuls cover the initial input-DMA window and
  b1's projections + chunked O-projections are interleaved into the
  exp-gated attention stream as filler to keep the PE dense.
- dma_starts cost ~0.7us of sequencer issue time each, so transfers are
  batched to ~100 total and spread across the sync/scalar/gpsimd queues.

Inputs are bf16 (host-side cast); accumulation is f32 in PSUM; the output
shard is written bf16 and upcast to f32 on the host.
"""

import numpy as np
import ml_dtypes

import concourse.bacc as bacc
import concourse.mybir as mybir
import concourse.tile as tile
from concourse.bass_utils import run_bass_kernel_spmd
from concourse.masks import make_identity

N_CORES = 8
B, T, C, H = 2, 2048, 1024, 16
D = 64                # head dim
HL = H // N_CORES     # heads per core = 2
DL = HL * D           # local feature dim = 128
TT = B * T            # 4096 tokens total
P = 128
NCH = C // P          # 8 contraction chunks
QCH = 512             # q-chunk (moving free dim)
NQC = T // QCH        # 4 q-chunks per batch entry
NKT = T // P          # 16 k-tiles per batch entry
HCH = T // 2          # AllGather chunk = half batch-entry = 1024 tokens
SCALE = 1.0 / np.sqrt(D)

BF = mybir.dt.bfloat16
F32 = mybir.dt.float32
AF = mybir.ActivationFunctionType


def build_graph():
    nc = bacc.Bacc("TRN2", target_bir_lowering=False, debug=False)

    xT = nc.dram_tensor("xT", [C, TT], BF, kind="ExternalInput")
    # all 4 weight shards pre-packed host-side into SBUF layout
    # [p, w, ci, m]: contiguous rows, loaded as 8 parallel DMAs
    wall = nc.dram_tensor("wall", [P, 4 * NCH * DL], BF, kind="ExternalInput")
    out = nc.dram_tensor("out", [DL, TT], BF, kind="ExternalOutput")

    with tile.TileContext(nc) as tc:
        with (
            tc.tile_pool(name="sb", bufs=1) as sb,
            tc.tile_pool(name="ps", bufs=1, space="PSUM") as ps,
            tc.tile_pool(name="dram", bufs=1, space="DRAM") as dram,
        ):
            # ---- loads ----
            w_sb = sb.tile([P, 4 * NCH * DL], BF, name="w_sb")
            WCOLS = 4 * NCH * DL
            for pc in range(8):
                csl = slice(pc * (WCOLS // 8), (pc + 1) * (WCOLS // 8))
                nc.sync.dma_start(w_sb[:, csl], wall[:, csl])
            w4 = w_sb[:].rearrange("p (w a m) -> p w a m", w=4, a=NCH)
            wq_sb, wk_sb, wv_sb, wo_sb = (w4[:, i] for i in range(4))

            ident = sb.tile([P, P], BF, name="ident")
            make_identity(nc, ident)
            # strictly-lower-triangular -1e9 (k > q): masks causal logits on
            # diagonal blocks, injected into the St PSUM group via
            # matmul(ident, mneg)
            mneg = sb.tile([P, P], BF, name="mneg")
            nc.gpsimd.memset(mneg[:], 0.0)
            nc.gpsimd.affine_select(
                out=mneg[:], in_=mneg[:],
                compare_op=mybir.AluOpType.is_ge,
                fill=-1e9, base=0, channel_multiplier=-1, pattern=[[1, P]],
            )
            wsrc = sb.tile([P, QCH], BF, name="wsrc")
            nc.vector.memset(wsrc[:], 0.5)
            for _ in range(22):
                wdst = ps.tile([P, QCH], F32, tag="st", bufs=2, name="wdst")
                nc.tensor.matmul(wdst[:], ident[:], wsrc[:],
                                 start=True, stop=True)

            qT_sb = sb.tile([P, TT], BF, name="qT_sb")
            kT_sb = sb.tile([P, TT], BF, name="kT_sb")
            vT_sb = sb.tile([P, TT], BF, name="vT_sb")
            # v in natural layout, packed per 128-token tile as
            # [headA(64) | 1 | headB(64) | 1] -> 130 columns
            v_sb = sb.tile([P, TT // P, 2 * (D + 1)], BF, name="v_sb")
            nc.gpsimd.memset(v_sb[:], 1.0)

            CHUNKS = [(c * QCH, QCH) for c in range(8)]
            ag_in = [
                dram.tile([DL, cw], BF, name=f"ag_in{c}")
                for c, (c0, cw) in enumerate(CHUNKS)
            ]
            ytf = [
                dram.tile([C, cw], BF, name=f"ytf{c}", addr_space="Shared")
                for c, (c0, cw) in enumerate(CHUNKS)
            ]
            # (b, jq) -> (chunk, col offset within chunk)
            CHUNK_OF = {(b, jq): (b * NQC + jq, 0)
                        for b in range(B) for jq in range(NQC)}

            with tc.tile_pool(name="xp", bufs=1) as xp:
                xT_sb = xp.tile([P, NCH, TT], BF, name="xT_sb")
                # first 512 columns per chunk land fast (small DMAs), the
                # rest streams as one big DMA per chunk; issued from the
                # scalar queue so the sync queue isn't serialized at start
                for ci in range(NCH):
                    nc.scalar.dma_start(
                        xT_sb[:, ci, 0:QCH], xT[ci * P:(ci + 1) * P, 0:QCH]
                    )
                for s0, s1 in ((QCH, 2 * QCH), (2 * QCH, 4 * QCH),
                               (4 * QCH, TT)):
                    for ci in range(NCH):
                        nc.scalar.dma_start(
                            xT_sb[:, ci, s0:s1], xT[ci * P:(ci + 1) * P, s0:s1]
                        )

                def proj_group(tch, wsb, dst):
                    tsl = slice(tch * QCH, (tch + 1) * QCH)
                    pj = ps.tile([P, QCH], F32, tag="st", bufs=2,
                                 name="pj")
                    for ci in range(NCH):
                        nc.tensor.matmul(
                            pj[:], wsb[:, ci, :], xT_sb[:, ci, tsl],
                            start=(ci == 0), stop=(ci == NCH - 1),
                        )
                    nc.vector.tensor_copy(dst[:, tsl], pj[:])

                def vtrans(t32):
                    tr = ps.tile([P, P], BF, tag="st", bufs=2, name="tr")
                    nc.tensor.transpose(
                        tr[:], vT_sb[:, t32 * P:(t32 + 1) * P], ident[:]
                    )
                    out_ap = v_sb[:, t32, :].rearrange(
                        "p (h x) -> p h x", h=HL
                    )[:, :, 0:D]
                    in_ap = tr[:].rearrange("p (h x) -> p h x", h=HL)
                    nc.vector.tensor_copy(out_ap, in_ap)

                def attn_compute(b, jq, h):
                    rsl = slice(h * D, (h + 1) * D)
                    q0 = b * T + jq * QCH
                    yt = ps.tile([D + 1, QCH], F32, tag="yt", bufs=2,
                                 name="yt")
                    nkt = 4 * jq + 4
                    for pr in range(nkt // 2):
                        st = ps.tile([P, 2 * QCH], F32, tag="st", bufs=2,
                                     name="st")
                        pt = sb.tile([P, 2 * QCH], BF, tag="pt", bufs=4,
                                     name="pt")
                        for half in range(2):
                            kt = 2 * pr + half
                            k0 = b * T + kt * P
                            i = kt - 4 * jq
                            # diagonal tiles: only q >= kt*128 live; leading
                            # 128 live columns get the -1e9 triangle
                            qv = max(i, 0) * P
                            ssl = slice(half * QCH + qv, (half + 1) * QCH)
                            nc.tensor.matmul(
                                st[:, ssl],
                                kT_sb[rsl, k0:k0 + P],
                                qT_sb[rsl, q0 + qv:q0 + QCH],
                                start=True, stop=(i < 0),
                            )
                            if i >= 0:
                                nc.tensor.matmul(
                                    st[:, half * QCH + qv:
                                       half * QCH + qv + P],
                                    ident[:], mneg[:],
                                    start=False, stop=True,
                                )
                        qv0 = max(2 * pr - 4 * jq, 0) * P
                        nc.scalar.activation(
                            pt[:, qv0:], st[:, qv0:], AF.Exp,
                            scale=float(SCALE)
                        )
                        for half in range(2):
                            kt = 2 * pr + half
                            qv = max(kt - 4 * jq, 0) * P
                            nc.tensor.matmul(
                                yt[:, qv:QCH],
                                v_sb[:, b * NKT + kt,
                                     h * (D + 1):(h + 1) * (D + 1)],
                                pt[:, half * QCH + qv:(half + 1) * QCH],
                                start=(kt == 0), stop=(kt == nkt - 1),
                            )
                    # denominator row -> SBUF bf16 right away; the rest of
                    # the eviction runs after the next filler block so the
                    # PE queue never waits on it
                    den = sb.tile([1, QCH], F32, tag="den", bufs=4, name="den")
                    nc.vector.tensor_copy(den[:], yt[D:D + 1, :])
                    return yt, den

                def attn_evict(b, jq, h, yt, den):
                    rsl = slice(h * D, (h + 1) * D)
                    bc = sb.tile([D, QCH], F32, tag="bc", bufs=3, name="bc")
                    nc.gpsimd.partition_broadcast(bc[:], den[:])
                    rcp = sb.tile([D, QCH], F32, tag="rcp", bufs=3, name="rcp")
                    scr = sb.tile([D, QCH], F32, tag="scr", bufs=3, name="scr")
                    nc.vector.reciprocal_approx_accurate(
                        rcp[:], bc[:], scratch=scr[:]
                    )
                    yn = sb.tile([D, QCH], BF, tag="yn", bufs=4, name="yn")
                    nc.vector.tensor_mul(yn[:], yt[0:D, :], rcp[:])
                    # stream this piece straight into the gather input;
                    # the final pieces are split for latency
                    c, off = CHUNK_OF[(b, jq)]
                    nsp = 2 if (b, jq) == (1, 3) else 1  # final pieces split for latency
                    w = QCH // nsp
                    for s in range(nsp):
                        nc.gpsimd.dma_start(
                            ag_in[c][h * D:(h + 1) * D,
                                     off + s * w:off + (s + 1) * w],
                            yn[:, s * w:(s + 1) * w],
                        )

                def ag_fire(c):
                    nc.gpsimd.collective_compute(
                        "AllGather",
                        mybir.AluOpType.bypass,
                        replica_groups=[list(range(N_CORES))],
                        ins=[ag_in[c][:]],
                        outs=[ytf[c][:]],
                    )

                yf_tiles = {}

                def yf_load(c):
                    c0, cw = CHUNKS[c]
                    yf = sb.tile([P, NCH, QCH], BF, tag="yf", bufs=2,
                                 name="yf")
                    yf_tiles[c] = yf
                    nsp = 2 if c >= len(CHUNKS) - 2 else 1
                    w = QCH // nsp
                    for ci in range(NCH):
                        for s in range(nsp):
                            nc.sync.dma_start(
                                yf[:, ci, s * w:(s + 1) * w],
                                ytf[c][ci * P:(ci + 1) * P,
                                       s * w:(s + 1) * w],
                            )

                def po_group(c, last=False):
                    c0, cw = CHUNKS[c]
                    yf = yf_tiles[c]
                    po = ps.tile([P, QCH], F32, tag="st", bufs=2, name="po")
                    for ci in range(NCH):
                        nc.tensor.matmul(
                            po[:], wo_sb[:, ci, :],
                            yf[:, ci, :],
                            start=(ci == 0), stop=(ci == NCH - 1),
                        )
                    ob = sb.tile([P, QCH], BF, tag="ob", bufs=2, name="ob")
                    nc.vector.tensor_copy(ob[:], po[:])
                    nsp = 4 if last else 1
                    w = QCH // nsp
                    for s in range(nsp):
                        o0 = c0 + s * w
                        nc.sync.dma_start(
                            out[:, o0:o0 + w], ob[:, s * w:(s + 1) * w]
                        )

                # ---- prologue: projections for b0 + b1's first chunk ----
                for tch in range(5):
                    for wsb, dst in ((wq_sb, qT_sb), (wk_sb, kT_sb),
                                     (wv_sb, vT_sb)):
                        proj_group(tch, wsb, dst)
                    for t32 in range(tch * 4, tch * 4 + 4):
                        vtrans(t32)

                # filler queues: b1 projections spread just-in-time so both
                # batches' exp-gated attention streams stay PE-dense.
                # tch5 is needed by b1 jq1, tch6 by jq2, tch7 by jq3.
                def proj_items(tchs):
                    items = []
                    for tch in tchs:
                        for wsb, dst in ((wq_sb, qT_sb), (wk_sb, kT_sb),
                                         (wv_sb, vT_sb)):
                            items.append((proj_group, (tch, wsb, dst)))
                        for t32 in range(tch * 4, tch * 4 + 4):
                            items.append((vtrans, (t32,)))
                    return items

                filler = proj_items([5, 6])
                def pop_filler(n):
                    for _ in range(min(n, len(filler))):
                        fn, args = filler.pop(0)
                        fn(*args)

                # ---- b0 attention ----
                for jq in range(NQC):
                    for h in range(HL):
                        yt, den = attn_compute(0, jq, h)
                        pop_filler(jq + 1)
                        if (jq, h) == (3, 0):
                            yf_load(0); po_group(0)
                        attn_evict(0, jq, h, yt, den)
                    pop_filler(1)
                    ag_fire(jq)
                pop_filler(99)

                # ---- b1 attention: tch7 projections + O-proj as filler ----
                filler = proj_items([7])
                for jq in range(NQC):
                    for h in range(HL):
                        yt, den = attn_compute(1, jq, h)
                        step = (jq, h)
                        if step == (0, 0):
                            yf_load(1); po_group(1)
                            pop_filler(3)
                        elif step == (0, 1):
                            pop_filler(2)
                        elif step == (1, 0):
                            yf_load(2); po_group(2)
                            pop_filler(2)
                        elif step == (1, 1):
                            pop_filler(99)
                        elif step == (2, 0):
                            yf_load(3); po_group(3)
                        elif step == (2, 1):
                            yf_load(4); po_group(4)
                        elif step == (3, 0):
                            yf_load(5); po_group(5)
                        attn_evict(1, jq, h, yt, den)
                    ag_fire(NQC + jq)
                # keep-warm matmuls: the PE would otherwise idle (and HAM-
                # throttle) while the final gathers fly, making the last
                # O-projections run at cold clock
                for _ in range(8):
                    wdst = ps.tile([P, QCH], F32, tag="st", bufs=2,
                                   name="wdst")
                    nc.tensor.matmul(wdst[:], ident[:], wsrc[:],
                                     start=True, stop=True)
                yf_load(6)
                po_group(6, last=True)
                for _ in range(24):
                    wdst = ps.tile([P, QCH], F32, tag="st", bufs=2,
                                   name="wdst")
                    nc.tensor.matmul(wdst[:], ident[:], wsrc[:],
                                     start=True, stop=True)
                yf_load(7)
                po_group(7, last=True)

    nc.finalize()
    return nc


_GRAPH = None


def _get_graph():
    global _GRAPH
    if _GRAPH is None:
        _GRAPH = build_graph()
    return _GRAPH


def prepare_in_maps(x, Wq, Wk, Wv, Wo):
    x = np.asarray(x, np.float32)
    Wq = np.asarray(Wq, np.float32)
    Wk = np.asarray(Wk, np.float32)
    Wv = np.asarray(Wv, np.float32)
    Wo = np.asarray(Wo, np.float32)

    bf = ml_dtypes.bfloat16
    xTh = np.ascontiguousarray(x.reshape(TT, C).T).astype(bf)
    in_maps = []
    for r in range(N_CORES):
        sl = slice(r * DL, (r + 1) * DL)
        # pack the 4 transposed weight shards into the SBUF layout
        # [p, w, ci, m] where the shard row index is c = ci*128 + p
        wall = np.empty((P, 4, NCH, DL), np.float32)
        for w, W in enumerate((Wq, Wk, Wv, Wo)):
            wall[:, w] = W[sl].T.reshape(NCH, P, DL).transpose(1, 0, 2)
        in_maps.append({
            "xT": xTh,
            "wall": np.ascontiguousarray(
                wall.reshape(P, 4 * NCH * DL)).astype(bf),
        })
    return in_maps


def assemble_output(results):
    outT = np.concatenate(
        [np.asarray(results[r]["out"], np.float32) for r in range(N_CORES)],
        axis=0,
    )  # [C, TT]
    return np.ascontiguousarray(outT.T).reshape(B, T, C)


def kernel(x, Wq, Wk, Wv, Wo):
    nc = _get_graph()
    in_maps = prepare_in_maps(x, Wq, Wk, Wv, Wo)
    res = run_bass_kernel_spmd(nc, in_maps, core_ids=list(range(N_CORES)))
    return assemble_output(res.results)



# revision 5
# speedup vs baseline: 1.0932x; 1.0932x over previous
"""Causal multi-head attention block (B=2, T=2048, C=1024, H=16) on 8 TRN2
NeuronCores.

Sharding: Megatron-style tensor parallel over heads. Core r owns heads
{2r, 2r+1} (output dims [128r, 128r+128) of Wq/Wk/Wv). The final output
projection contracts over all of C, so cores AllGather their local yT
shards (concat on the partition axis == feature axis) into yT_full
[C, B*T], then each core computes a 128-column shard of the output:
outT_shard = Wo[128r:128r+128, :] @ yT_full.

Everything on-device is computed in the "transposed" orientation
(feature-major, token-minor) so the TensorEngine contraction axis always
sits on SBUF partitions and the softmax denominator arrives for free via
a ones-column appended to V:

  qT/kT/vT [128, 4096] = W_shard @ x^T          (x^T passed from host)
  ST tile [128k, 512q] = kT_slice.T @ qT_slice  (contract d=64)
  causal mask: add a -1e9 strictly-lower-triangular matrix into the St
      PSUM accumulation group via matmul(ident, mneg) on diagonal blocks
  PT = exp(ST * 1/sqrt(d))                      (no max-subtraction: logits
                                                 are ~N(0,1), |S|max ~ 6)
  yT [65, 512] += [v | 1].T @ PT                (row 64 = softmax denom)
  yT_norm = yT[0:64] * partition_broadcast(recip(yT[64]))

Performance structure (v2, from perfetto/HAM analysis of v1):
- The exp stream on ACT is the kernel's clock (~1.15us per 1024-col
  ACTIVATE); the PE must stay dense to hold HAM at K=8/8 (2.4 GHz).
- Attention starts as soon as tch0's projections land (~12us), not after
  the whole b0 projection prologue; every other projection group and the
  chunked O-projection are split into single-matmul filler items popped
  between the exp-gated pairs (2-4 per pair), keeping the PE dense
  through the entire exp stream.
- The scalar (ACT) queue carries ONLY the exps plus 4 batched xT input
  DMA issues at t=0 (v1 had 32 issues spanning 11-40us there, delaying
  the exp stream).  A single dma_start fans across all 16 SDMA engines,
  so batching costs no bandwidth.
- PSUM tags are split (st 2x2 banks, yt 2x1, aux 2x1 for
  proj/O-proj/transpose/warmup) so filler matmul groups never
  false-share a PSUM slot with the score tiles.
- Per-(b,jq) yT chunks AllGather as soon as both heads evict; the last
  chunk is split per head (two [64,512] gathers) so the final gather
  fires one head earlier and carries half the bytes.  A dummy 1-element
  AllGather at t=0 absorbs the collective-stack init latency.
- O-proj chunk c is consumed as filler two jq-slots after its gather
  fires (gather latency ~8-15us); its yf load is one batched dma_start.

Inputs are bf16 (host-side cast); accumulation is f32 in PSUM; the output
shard is written bf16 and upcast to f32 on the host.
"""

import numpy as np
import ml_dtypes

import concourse.bacc as bacc
import concourse.mybir as mybir
import concourse.tile as tile
from concourse.bass_utils import run_bass_kernel_spmd
from concourse.masks import make_identity

N_CORES = 8
B, T, C, H = 2, 2048, 1024, 16
D = 64                # head dim
HL = H // N_CORES     # heads per core = 2
DL = HL * D           # local feature dim = 128
TT = B * T            # 4096 tokens total
P = 128
NCH = C // P          # 8 contraction chunks
QCH = 512             # q-chunk (moving free dim)
NQC = T // QCH        # 4 q-chunks per batch entry
NKT = T // P          # 16 k-tiles per batch entry
SCALE = 1.0 / np.sqrt(D)

BF = mybir.dt.bfloat16
F32 = mybir.dt.float32
AF = mybir.ActivationFunctionType


def build_graph():
    nc = bacc.Bacc("TRN2", target_bir_lowering=False, debug=False)

    xT = nc.dram_tensor("xT", [C, TT], BF, kind="ExternalInput")
    # all 4 weight shards pre-packed host-side into SBUF layout
    # [p, w, ci, m]: contiguous rows
    wall = nc.dram_tensor("wall", [P, 4 * NCH * DL], BF, kind="ExternalInput")
    out = nc.dram_tensor("out", [DL, TT], BF, kind="ExternalOutput")

    with tile.TileContext(nc) as tc:
        with (
            tc.tile_pool(name="sb", bufs=1) as sb,
            tc.tile_pool(name="ps", bufs=1, space="PSUM") as ps,
            tc.tile_pool(name="dram", bufs=1, space="DRAM") as dram,
        ):
            # ---- input loads ----
            w_sb = sb.tile([P, 4 * NCH * DL], BF, name="w_sb")
            WCOLS = 4 * NCH * DL
            for pc in range(8):
                csl = slice(pc * (WCOLS // 8), (pc + 1) * (WCOLS // 8))
                nc.sync.dma_start(w_sb[:, csl], wall[:, csl])
            w4 = w_sb[:].rearrange("p (w a m) -> p w a m", w=4, a=NCH)
            wq_sb, wk_sb, wv_sb, wo_sb = (w4[:, i] for i in range(4))

            # xT in [p, ci, t] layout; 4 batched issues on the scalar
            # queue (done issuing ~2us in, long before the first exp)
            xT_sb = sb.tile([P, NCH, TT], BF, name="xT_sb")
            xTr = xT[:, :].rearrange("(a p) t -> p a t", p=P)
            for s0, s1 in ((0, QCH), (QCH, 2 * QCH), (2 * QCH, 4 * QCH),
                           (4 * QCH, TT)):
                nc.scalar.dma_start(xT_sb[:, :, s0:s1], xTr[:, :, s0:s1])

            # dummy collective to absorb comm-stack init before the
            # first real gather
            agw_in = dram.tile([64, 8], BF, name="agw_in")
            agw_out = dram.tile([512, 8], BF, name="agw_out",
                                addr_space="Shared")
            nc.gpsimd.collective_compute(
                "AllGather", mybir.AluOpType.bypass,
                replica_groups=[list(range(N_CORES))],
                ins=[agw_in[:]], outs=[agw_out[:]],
            )

            ident = sb.tile([P, P], BF, name="ident")
            make_identity(nc, ident)
            # strictly-lower-triangular -1e9 (k > q): masks causal logits
            # on diagonal blocks via matmul(ident, mneg)
            mneg = sb.tile([P, P], BF, name="mneg")
            nc.gpsimd.memset(mneg[:], 0.0)
            nc.gpsimd.affine_select(
                out=mneg[:], in_=mneg[:],
                compare_op=mybir.AluOpType.is_ge,
                fill=-1e9, base=0, channel_multiplier=-1, pattern=[[1, P]],
            )
            wsrc = sb.tile([P, QCH], BF, name="wsrc")
            nc.vector.memset(wsrc[:], 0.5)

            def warm(n):
                for _ in range(n):
                    wdst = ps.tile([P, QCH], F32, tag="aux", bufs=2,
                                   name="wdst")
                    nc.tensor.matmul(wdst[:], ident[:], wsrc[:],
                                     start=True, stop=True)

            warm(14)

            qT_sb = sb.tile([P, TT], BF, name="qT_sb")
            kT_sb = sb.tile([P, TT], BF, name="kT_sb")
            vT_sb = sb.tile([P, TT], BF, name="vT_sb")
            # v in natural layout, packed per 128-token tile as
            # [headA(64) | 1 | headB(64) | 1] -> 130 columns
            v_sb = sb.tile([P, TT // P, 2 * (D + 1)], BF, name="v_sb")
            nc.gpsimd.memset(v_sb[:], 1.0)

            # ---- AllGather buffers ----
            # chunks 0..6: per-(b,jq) [128, 512]; chunk 7 split per head
            ag_in = [dram.tile([DL, QCH], BF, name=f"ag_in{c}")
                     for c in range(7)]
            ag7 = [dram.tile([D, QCH], BF, name=f"ag7_{h}")
                   for h in range(HL)]
            ytf = [dram.tile([C, QCH], BF, name=f"ytf{c}",
                             addr_space="Shared") for c in range(7)]
            ytf7 = [dram.tile([C // 2, QCH], BF, name=f"ytf7_{h}",
                              addr_space="Shared") for h in range(HL)]

            # ---- projection / transpose / O-proj work items ----
            def make_proj_items(tch, wsb, dst):
                tsl = slice(tch * QCH, (tch + 1) * QCH)
                state = {}
                items = []
                for ci in range(NCH):
                    def mm(ci=ci, tsl=tsl, state=state, wsb=wsb, dst=dst):
                        if ci == 0:
                            state['pj'] = ps.tile([P, QCH], F32, tag="aux",
                                                  bufs=2, name="pj")
                        nc.tensor.matmul(
                            state['pj'][:], wsb[:, ci, :], xT_sb[:, ci, tsl],
                            start=(ci == 0), stop=(ci == NCH - 1),
                        )
                        if ci == NCH - 1:
                            nc.vector.tensor_copy(dst[:, tsl], state['pj'][:])
                    items.append(mm)
                return items

            def make_vtrans_item(t32):
                def it(t32=t32):
                    tr = ps.tile([P, P], BF, tag="aux", bufs=2, name="tr")
                    nc.tensor.transpose(
                        tr[:], vT_sb[:, t32 * P:(t32 + 1) * P], ident[:]
                    )
                    out_ap = v_sb[:, t32, :].rearrange(
                        "p (h x) -> p h x", h=HL
                    )[:, :, 0:D]
                    in_ap = tr[:].rearrange("p (h x) -> p h x", h=HL)
                    nc.vector.tensor_copy(out_ap, in_ap)
                return it

            def proj_items(tch):
                items = []
                for wsb, dst in ((wq_sb, qT_sb), (wk_sb, kT_sb),
                                 (wv_sb, vT_sb)):
                    items.extend(make_proj_items(tch, wsb, dst))
                for t32 in range(tch * 4, tch * 4 + 4):
                    items.append(make_vtrans_item(t32))
                return items

            def yf_load(c):
                yf = sb.tile([P, NCH, QCH], BF, tag="yf", bufs=2, name="yf")
                if c < 7:
                    src = ytf[c][:, :].rearrange("(a p) t -> p a t", p=P)
                    nc.sync.dma_start(yf[:, :, :], src)
                else:
                    for h in range(HL):
                        src = ytf7[h][:, :].rearrange("(a p) t -> p a t",
                                                      p=D)
                        nc.sync.dma_start(yf[h * D:(h + 1) * D, :, :], src)
                return yf

            def make_po_items(c, yf, last=False):
                c0 = c * QCH
                state = {}
                items = []
                for ci in range(NCH):
                    def mm(ci=ci, state=state, yf=yf, c0=c0, last=last):
                        if ci == 0:
                            state['po'] = ps.tile([P, QCH], F32, tag="aux",
                                                  bufs=2, name="po")
                        nc.tensor.matmul(
                            state['po'][:], wo_sb[:, ci, :], yf[:, ci, :],
                            start=(ci == 0), stop=(ci == NCH - 1),
                        )
                        if ci == NCH - 1:
                            ob = sb.tile([P, QCH], BF, tag="ob", bufs=2,
                                         name="ob")
                            nc.vector.tensor_copy(ob[:], state['po'][:])
                            nsp = 4 if last else 1
                            w = QCH // nsp
                            for s in range(nsp):
                                nc.sync.dma_start(
                                    out[:, c0 + s * w:c0 + (s + 1) * w],
                                    ob[:, s * w:(s + 1) * w],
                                )
                    items.append(mm)
                return items

            # ---- filler machinery ----
            # FIFO of (key, fn): key = tch index for projection items
            # (force-drained before the slot that needs them), 99 for
            # O-proj items (no deadline).  Strict FIFO execution keeps
            # the aux-PSUM group invariant (<= 2 open matmul groups).
            filler = []

            def pop_filler(n):
                for _ in range(min(n, len(filler))):
                    filler.pop(0)[1]()

            def drain_tch(tmax):
                while any(k <= tmax for k, _ in filler):
                    filler.pop(0)[1]()

            def drain_filler():
                while filler:
                    filler.pop(0)[1]()

            # ---- attention ----
            def attn_compute(b, jq, h, per_pair=2):
                rsl = slice(h * D, (h + 1) * D)
                q0 = b * T + jq * QCH
                yt = ps.tile([D + 1, QCH], F32, tag="yt", bufs=2, name="yt")
                nkt = 4 * jq + 4
                for pr in range(nkt // 2):
                    st = ps.tile([P, 2 * QCH], F32, tag="st", bufs=2,
                                 name="st")
                    pt = sb.tile([P, 2 * QCH], BF, tag="pt", bufs=4,
                                 name="pt")
                    for half in range(2):
                        kt = 2 * pr + half
                        k0 = b * T + kt * P
                        i = kt - 4 * jq
                        # diagonal tiles: only q >= kt*128 live; leading
                        # 128 live columns get the -1e9 triangle
                        qv = max(i, 0) * P
                        ssl = slice(half * QCH + qv, (half + 1) * QCH)
                        nc.tensor.matmul(
                            st[:, ssl],
                            kT_sb[rsl, k0:k0 + P],
                            qT_sb[rsl, q0 + qv:q0 + QCH],
                            start=True, stop=(i < 0),
                        )
                        if i >= 0:
                            nc.tensor.matmul(
                                st[:, half * QCH + qv:
                                   half * QCH + qv + P],
                                ident[:], mneg[:],
                                start=False, stop=True,
                            )
                    qv0 = max(2 * pr - 4 * jq, 0) * P
                    nc.scalar.activation(
                        pt[:, qv0:], st[:, qv0:], AF.Exp,
                        scale=float(SCALE)
                    )
                    # PE filler between the exp issue and the AV matmuls:
                    # the PE chews these while ACT computes the exp
                    pop_filler(per_pair)
                    for half in range(2):
                        kt = 2 * pr + half
                        qv = max(kt - 4 * jq, 0) * P
                        nc.tensor.matmul(
                            yt[:, qv:QCH],
                            v_sb[:, b * NKT + kt,
                                 h * (D + 1):(h + 1) * (D + 1)],
                            pt[:, half * QCH + qv:(half + 1) * QCH],
                            start=(kt == 0), stop=(kt == nkt - 1),
                        )
                # denominator row -> SBUF f32 right away
                den = sb.tile([1, QCH], F32, tag="den", bufs=4, name="den")
                nc.vector.tensor_copy(den[:], yt[D:D + 1, :])
                return yt, den

            def attn_evict(b, jq, h, yt, den):
                bc = sb.tile([D, QCH], F32, tag="bc", bufs=3, name="bc")
                nc.gpsimd.partition_broadcast(bc[:], den[:])
                rcp = sb.tile([D, QCH], F32, tag="rcp", bufs=3, name="rcp")
                scr = sb.tile([D, QCH], F32, tag="scr", bufs=3, name="scr")
                nc.vector.reciprocal_approx_accurate(
                    rcp[:], bc[:], scratch=scr[:]
                )
                yn = sb.tile([D, QCH], BF, tag="yn", bufs=4, name="yn")
                nc.vector.tensor_mul(yn[:], yt[0:D, :], rcp[:])
                c = b * NQC + jq
                if c < 7:
                    nc.gpsimd.dma_start(
                        ag_in[c][h * D:(h + 1) * D, :], yn[:, :]
                    )
                else:
                    nc.gpsimd.dma_start(ag7[h][:, :], yn[:, :])

            def ag_fire(c):
                nc.gpsimd.collective_compute(
                    "AllGather", mybir.AluOpType.bypass,
                    replica_groups=[list(range(N_CORES))],
                    ins=[ag_in[c][:]], outs=[ytf[c][:]],
                )

            def ag_fire7(h):
                nc.gpsimd.collective_compute(
                    "AllGather", mybir.AluOpType.bypass,
                    replica_groups=[list(range(N_CORES))],
                    ins=[ag7[h][:]], outs=[ytf7[h][:]],
                )

            # ---- prologue: tch0 projections only ----
            for wsb, dst in ((wq_sb, qT_sb), (wk_sb, kT_sb), (wv_sb, vT_sb)):
                tsl = slice(0, QCH)
                pj = ps.tile([P, QCH], F32, tag="aux", bufs=2, name="pj")
                for ci in range(NCH):
                    nc.tensor.matmul(
                        pj[:], wsb[:, ci, :], xT_sb[:, ci, tsl],
                        start=(ci == 0), stop=(ci == NCH - 1),
                    )
                nc.vector.tensor_copy(dst[:, tsl], pj[:])
            for t32 in range(4):
                make_vtrans_item(t32)()

            # seed filler with the next projections
            filler.extend((1, it) for it in proj_items(1))
            filler.extend((2, it) for it in proj_items(2))

            # ---- main loop over (b, jq) ----
            # slot s = b*NQC + jq uses projection chunk tch=s (tokens
            # [s*512,(s+1)*512)); tch s+2 is appended to the filler at
            # slot s.  O-proj for gather chunk c enters the filler two
            # slots after its AllGather fires (gather latency ~8-15us).
            SLOTS = [(b, jq) for b in range(B) for jq in range(NQC)]

            for s, (b, jq) in enumerate(SLOTS):
                if s >= 1:
                    drain_tch(s)  # projections this slot reads
                    if s + 2 < len(SLOTS):
                        filler.extend((s + 2, it)
                                      for it in proj_items(s + 2))
                if s >= 2:
                    c = s - 2
                    yf = yf_load(c)
                    filler.extend((99, it) for it in make_po_items(c, yf))
                    if s == 7:
                        yf6 = yf_load(6)
                        filler.extend((99, it)
                                      for it in make_po_items(6, yf6))
                # pace filler so items due next slot drain this slot;
                # surplus beyond that drains at >=2/pair
                npairs = 4 * (jq + 1)
                due = sum(1 for k, _ in filler if k <= s + 1)
                per_pair = max(2, min(6, -(-due // npairs)))
                for h in range(HL):
                    yt, den = attn_compute(b, jq, h, per_pair=per_pair)
                    attn_evict(b, jq, h, yt, den)
                    if s == len(SLOTS) - 1:
                        ag_fire7(h)
                if s < len(SLOTS) - 1:
                    ag_fire(s)

            # ---- tail: drain leftovers, keep PE warm, last O-proj ----
            drain_filler()
            warm(12)
            yf7 = yf_load(7)
            for it in make_po_items(7, yf7, last=True):
                it()

    nc.finalize()
    return nc


_GRAPH = None


def _get_graph():
    global _GRAPH
    if _GRAPH is None:
        _GRAPH = build_graph()
    return _GRAPH


def prepare_in_maps(x, Wq, Wk, Wv, Wo):
    x = np.asarray(x, np.float32)
    Wq = np.asarray(Wq, np.float32)
    Wk = np.asarray(Wk, np.float32)
    Wv = np.asarray(Wv, np.float32)
    Wo = np.asarray(Wo, np.float32)

    bf = ml_dtypes.bfloat16
    xTh = np.ascontiguousarray(x.reshape(TT, C).T).astype(bf)
    in_maps = []
    for r in range(N_CORES):
        sl = slice(r * DL, (r + 1) * DL)
        # pack the 4 transposed weight shards into the SBUF layout
        # [p, w, ci, m] where the shard row index is c = ci*128 + p
        wall = np.empty((P, 4, NCH, DL), np.float32)
        for w, W in enumerate((Wq, Wk, Wv, Wo)):
            wall[:, w] = W[sl].T.reshape(NCH, P, DL).transpose(1, 0, 2)
        in_maps.append({
            "xT": xTh,
            "wall": np.ascontiguousarray(
                wall.reshape(P, 4 * NCH * DL)).astype(bf),
        })
    return in_maps


def assemble_output(results):
    outT = np.concatenate(
        [np.asarray(results[r]["out"], np.float32) for r in range(N_CORES)],
        axis=0,
    )  # [C, TT]
    return np.ascontiguousarray(outT.T).reshape(B, T, C)


def kernel(x, Wq, Wk, Wv, Wo):
    nc = _get_graph()
    in_maps = prepare_in_maps(x, Wq, Wk, Wv, Wo)
    res = run_bass_kernel_spmd(nc, in_maps, core_ids=list(range(N_CORES)))
    return assemble_output(res.results)


# revision 10
# speedup vs baseline: 1.1493x; 1.0513x over previous
"""Causal multi-head attention block (B=2, T=2048, C=1024, H=16) on 8 TRN2
NeuronCores.

Sharding: Megatron-style tensor parallel over heads. Core r owns heads
{2r, 2r+1} (output dims [128r, 128r+128) of Wq/Wk/Wv). The final output
projection contracts over all of C, so cores AllGather their local yT
shards (concat on the partition axis == feature axis) into yT_full
[C, B*T], then each core computes a 128-column shard of the output:
outT_shard = Wo[128r:128r+128, :] @ yT_full.

Everything on-device is computed in the "transposed" orientation
(feature-major, token-minor) so the TensorEngine contraction axis always
sits on SBUF partitions and the softmax denominator arrives for free via
a ones-column appended to V:

  qT/kT/vT [128, 4096] = W_shard @ x^T          (x^T passed from host)
  ST tile [128k, 512q] = kT_slice.T @ qT_slice  (contract d=64)
  causal mask: add a -1e9 strictly-lower-triangular matrix into the St
      PSUM accumulation group via matmul(ident, mneg) on diagonal blocks
  PT = exp(ST * 1/sqrt(d))                      (no max-subtraction: logits
                                                 are ~N(0,1), |S|max ~ 6)
  yT [65, 512] += [v | 1].T @ PT                (row 64 = softmax denom)
  yT_norm = yT[0:64] * partition_broadcast(recip(yT[64]))

Performance structure (v2, from perfetto/HAM analysis of v1):
- The exp stream on ACT is the kernel's clock (~1.15us per 1024-col
  ACTIVATE); the PE must stay dense to hold HAM at K=8/8 (2.4 GHz).
- Attention starts as soon as tch0's projections land (~12us), not after
  the whole b0 projection prologue; every other projection group and the
  chunked O-projection are split into single-matmul filler items popped
  between the exp-gated pairs (2-4 per pair), keeping the PE dense
  through the entire exp stream.
- The scalar (ACT) queue carries ONLY the exps plus 4 batched xT input
  DMA issues at t=0 (v1 had 32 issues spanning 11-40us there, delaying
  the exp stream).  A single dma_start fans across all 16 SDMA engines,
  so batching costs no bandwidth.
- PSUM tags are split (st 2x2 banks, yt 2x1, aux 2x1 for
  proj/O-proj/transpose/warmup) so filler matmul groups never
  false-share a PSUM slot with the score tiles.
- Per-(b,jq) yT chunks AllGather as soon as both heads evict; the last
  chunk is split per head (two [64,512] gathers) so the final gather
  fires one head earlier and carries half the bytes.  A dummy 1-element
  AllGather at t=0 absorbs the collective-stack init latency.
- O-proj chunk c is consumed as filler two jq-slots after its gather
  fires (gather latency ~8-15us); its yf load is one batched dma_start.

Inputs are bf16 (host-side cast); accumulation is f32 in PSUM; the output
shard is written bf16 and upcast to f32 on the host.
"""

import numpy as np
import ml_dtypes

import concourse.bacc as bacc
import concourse.mybir as mybir
import concourse.tile as tile
from concourse.bass_utils import run_bass_kernel_spmd
from concourse.masks import make_identity

N_CORES = 8
B, T, C, H = 2, 2048, 1024, 16
D = 64                # head dim
HL = H // N_CORES     # heads per core = 2
DL = HL * D           # local feature dim = 128
TT = B * T            # 4096 tokens total
P = 128
NCH = C // P          # 8 contraction chunks
QCH = 512             # q-chunk (moving free dim)
NQC = T // QCH        # 4 q-chunks per batch entry
NKT = T // P          # 16 k-tiles per batch entry
SCALE = 1.0 / np.sqrt(D)

BF = mybir.dt.bfloat16
F32 = mybir.dt.float32
AF = mybir.ActivationFunctionType


def build_graph():
    nc = bacc.Bacc("TRN2", target_bir_lowering=False, debug=False)

    xT = nc.dram_tensor("xT", [C, TT], BF, kind="ExternalInput")
    # all 4 weight shards pre-packed host-side into SBUF layout
    # [p, w, ci, m]: contiguous rows
    wall = nc.dram_tensor("wall", [P, 4 * NCH * DL], BF, kind="ExternalInput")
    out = nc.dram_tensor("out", [DL, TT], BF, kind="ExternalOutput")

    with tile.TileContext(nc) as tc:
        with (
            tc.tile_pool(name="sb", bufs=1) as sb,
            tc.tile_pool(name="ps", bufs=1, space="PSUM") as ps,
            tc.tile_pool(name="dram", bufs=1, space="DRAM") as dram,
        ):
            # ---- input loads ----
            w_sb = sb.tile([P, 4 * NCH * DL], BF, name="w_sb")
            WCOLS = 4 * NCH * DL
            for pc in range(8):
                csl = slice(pc * (WCOLS // 8), (pc + 1) * (WCOLS // 8))
                nc.sync.dma_start(w_sb[:, csl], wall[:, csl])
            w4 = w_sb[:].rearrange("p (w a m) -> p w a m", w=4, a=NCH)
            wq_sb, wk_sb, wv_sb, wo_sb = (w4[:, i] for i in range(4))

            # xT in [p, ci, t] layout; 4 batched issues on the scalar
            # queue (done issuing ~2us in, long before the first exp)
            xT_sb = sb.tile([P, NCH, TT], BF, name="xT_sb")
            xTr = xT[:, :].rearrange("(a p) t -> p a t", p=P)
            for s0, s1 in ((0, QCH), (QCH, 2 * QCH), (2 * QCH, 4 * QCH),
                           (4 * QCH, TT)):
                nc.scalar.dma_start(xT_sb[:, :, s0:s1], xTr[:, :, s0:s1])

            ident = sb.tile([P, P], BF, name="ident")
            make_identity(nc, ident)
            # strictly-lower-triangular -1e9 (k > q): masks causal logits
            # on diagonal blocks via matmul(ident, mneg)
            mneg = sb.tile([P, P], BF, name="mneg")
            nc.gpsimd.memset(mneg[:], 0.0)
            nc.gpsimd.affine_select(
                out=mneg[:], in_=mneg[:],
                compare_op=mybir.AluOpType.is_ge,
                fill=-1e9, base=0, channel_multiplier=-1, pattern=[[1, P]],
            )
            wsrc = sb.tile([P, QCH], BF, name="wsrc")
            nc.vector.memset(wsrc[:], 0.5)

            def warm(n):
                for _ in range(n):
                    wdst = ps.tile([P, QCH], F32, tag="aux", bufs=2,
                                   name="wdst")
                    nc.tensor.matmul(wdst[:], ident[:], wsrc[:],
                                     start=True, stop=True)

            warm(6)

            qT_sb = sb.tile([P, TT], BF, name="qT_sb")
            kT_sb = sb.tile([P, TT], BF, name="kT_sb")
            vT_sb = sb.tile([P, TT], BF, name="vT_sb")
            # v in natural layout, packed per 128-token tile as
            # [headA(64) | 1 | headB(64) | 1] -> 130 columns
            v_sb = sb.tile([P, TT // P, 2 * (D + 1)], BF, name="v_sb")
            nc.gpsimd.memset(v_sb[:], 1.0)

            # ---- AllGather buffers ----
            # The collective stream is serial and unavailable for the
            # first ~72us (comm-stack init), and each op costs
            # ~10-17us.  Four asymmetric gathers: all of b0 at once
            # (its data is ready while the stream is still in init),
            # then b1 in a 2-chunk piece and two 1-chunk pieces so the
            # last gather (which bounds the tail) is small.
            #   A: b0 jq0-3  [128, 2048]   B: b1 jq0-1 [128, 1024]
            #   C: b1 jq2    [128, 512]    D: b1 jq3   [128, 512]
            AG_NJQ = (4, 2, 1, 1)   # jq chunks per gather
            ag_in = [dram.tile([DL, n * QCH], BF, name=f"ag_in{g}")
                     for g, n in enumerate(AG_NJQ)]
            ytf = [dram.tile([C, n * QCH], BF, name=f"ytf{g}",
                             addr_space="Shared")
                   for g, n in enumerate(AG_NJQ)]
            # slot s -> (gather idx, jq offset within gather)
            AG_OF = {0: (0, 0), 1: (0, 1), 2: (0, 2), 3: (0, 3),
                     4: (1, 0), 5: (1, 1), 6: (2, 0), 7: (3, 0)}

            # ---- projection / transpose / O-proj work items ----
            def make_proj_items(tch, wsb, dst):
                tsl = slice(tch * QCH, (tch + 1) * QCH)
                state = {}
                items = []
                for ci in range(NCH):
                    def mm(ci=ci, tsl=tsl, state=state, wsb=wsb, dst=dst):
                        if ci == 0:
                            state['pj'] = ps.tile([P, QCH], F32, tag="aux",
                                                  bufs=2, name="pj")
                        nc.tensor.matmul(
                            state['pj'][:], wsb[:, ci, :], xT_sb[:, ci, tsl],
                            start=(ci == 0), stop=(ci == NCH - 1),
                        )
                        if ci == NCH - 1:
                            nc.vector.tensor_copy(dst[:, tsl], state['pj'][:])
                    items.append(mm)
                return items

            def make_vtrans_item(t32):
                def it(t32=t32):
                    tr = ps.tile([P, P], BF, tag="aux", bufs=2, name="tr")
                    nc.tensor.transpose(
                        tr[:], vT_sb[:, t32 * P:(t32 + 1) * P], ident[:]
                    )
                    out_ap = v_sb[:, t32, :].rearrange(
                        "p (h x) -> p h x", h=HL
                    )[:, :, 0:D]
                    in_ap = tr[:].rearrange("p (h x) -> p h x", h=HL)
                    nc.vector.tensor_copy(out_ap, in_ap)
                return it

            def proj_items(tch):
                items = []
                for wsb, dst in ((wq_sb, qT_sb), (wk_sb, kT_sb),
                                 (wv_sb, vT_sb)):
                    items.extend(make_proj_items(tch, wsb, dst))
                for t32 in range(tch * 4, tch * 4 + 4):
                    items.append(make_vtrans_item(t32))
                return items

            def yf_load(c):
                g, off = AG_OF[c]
                yf = sb.tile([P, NCH, QCH], BF, tag="yf", bufs=2, name="yf")
                src = ytf[g][:, off * QCH:(off + 1) * QCH].rearrange(
                    "(a p) t -> p a t", p=P)
                nc.sync.dma_start(yf[:, :, :], src)
                return yf

            def make_po_items(c, yf, last=False):
                c0 = c * QCH
                state = {}
                items = []
                for ci in range(NCH):
                    def mm(ci=ci, state=state, yf=yf, c0=c0, last=last):
                        if ci == 0:
                            state['po'] = ps.tile([P, QCH], F32, tag="aux",
                                                  bufs=2, name="po")
                        nc.tensor.matmul(
                            state['po'][:], wo_sb[:, ci, :], yf[:, ci, :],
                            start=(ci == 0), stop=(ci == NCH - 1),
                        )
                        if ci == NCH - 1:
                            ob = sb.tile([P, QCH], BF, tag="ob", bufs=2,
                                         name="ob")
                            nc.vector.tensor_copy(ob[:], state['po'][:])
                            nsp = 4 if last else 1
                            w = QCH // nsp
                            for s in range(nsp):
                                nc.sync.dma_start(
                                    out[:, c0 + s * w:c0 + (s + 1) * w],
                                    ob[:, s * w:(s + 1) * w],
                                )
                    items.append(mm)
                return items

            # ---- filler machinery ----
            # FIFO of (key, fn): key = tch index for projection items
            # (force-drained before the slot that needs them), 99 for
            # O-proj items (no deadline).  Strict FIFO execution keeps
            # the aux-PSUM group invariant (<= 2 open matmul groups).
            filler = []

            def pop_filler(n):
                for _ in range(min(n, len(filler))):
                    filler.pop(0)[1]()

            def drain_tch(tmax):
                while any(k <= tmax for k, _ in filler):
                    filler.pop(0)[1]()

            def drain_filler():
                while filler:
                    filler.pop(0)[1]()

            # ---- attention ----
            def attn_compute(b, jq, h, per_pair=2):
                rsl = slice(h * D, (h + 1) * D)
                q0 = b * T + jq * QCH
                yt = ps.tile([D + 1, QCH], F32, tag="yt", bufs=2, name="yt")
                nkt = 4 * jq + 4
                for pr in range(nkt // 2):
                    st = ps.tile([P, 2 * QCH], F32, tag="st", bufs=2,
                                 name="st")
                    pt = sb.tile([P, 2 * QCH], BF, tag="pt", bufs=4,
                                 name="pt")
                    for half in range(2):
                        kt = 2 * pr + half
                        k0 = b * T + kt * P
                        i = kt - 4 * jq
                        # diagonal tiles: only q >= kt*128 live; leading
                        # 128 live columns get the -1e9 triangle
                        qv = max(i, 0) * P
                        ssl = slice(half * QCH + qv, (half + 1) * QCH)
                        nc.tensor.matmul(
                            st[:, ssl],
                            kT_sb[rsl, k0:k0 + P],
                            qT_sb[rsl, q0 + qv:q0 + QCH],
                            start=True, stop=(i < 0),
                        )
                        if i >= 0:
                            nc.tensor.matmul(
                                st[:, half * QCH + qv:
                                   half * QCH + qv + P],
                                ident[:], mneg[:],
                                start=False, stop=True,
                            )
                    qv0 = max(2 * pr - 4 * jq, 0) * P
                    nc.scalar.activation(
                        pt[:, qv0:], st[:, qv0:], AF.Exp,
                        scale=float(SCALE)
                    )
                    # PE filler between the exp issue and the AV matmuls:
                    # the PE chews these while ACT computes the exp
                    pop_filler(per_pair)
                    for half in range(2):
                        kt = 2 * pr + half
                        qv = max(kt - 4 * jq, 0) * P
                        nc.tensor.matmul(
                            yt[:, qv:QCH],
                            v_sb[:, b * NKT + kt,
                                 h * (D + 1):(h + 1) * (D + 1)],
                            pt[:, half * QCH + qv:(half + 1) * QCH],
                            start=(kt == 0), stop=(kt == nkt - 1),
                        )
                # denominator row -> SBUF f32 right away
                den = sb.tile([1, QCH], F32, tag="den", bufs=4, name="den")
                nc.vector.tensor_copy(den[:], yt[D:D + 1, :])
                return yt, den

            def attn_evict(b, jq, h, yt, den):
                bc = sb.tile([D, QCH], F32, tag="bc", bufs=3, name="bc")
                nc.gpsimd.partition_broadcast(bc[:], den[:])
                rcp = sb.tile([D, QCH], F32, tag="rcp", bufs=3, name="rcp")
                scr = sb.tile([D, QCH], F32, tag="scr", bufs=3, name="scr")
                nc.vector.reciprocal_approx_accurate(
                    rcp[:], bc[:], scratch=scr[:]
                )
                yn = sb.tile([D, QCH], BF, tag="yn", bufs=4, name="yn")
                nc.vector.tensor_mul(yn[:], yt[0:D, :], rcp[:])
                g, off = AG_OF[b * NQC + jq]
                nc.gpsimd.dma_start(
                    ag_in[g][h * D:(h + 1) * D,
                             off * QCH:(off + 1) * QCH],
                    yn[:, :],
                )

            def ag_fire(g):
                nc.gpsimd.collective_compute(
                    "AllGather", mybir.AluOpType.bypass,
                    replica_groups=[list(range(N_CORES))],
                    ins=[ag_in[g][:]], outs=[ytf[g][:]],
                )

            # ---- prologue: tch0 projections only ----
            for wsb, dst in ((wq_sb, qT_sb), (wk_sb, kT_sb), (wv_sb, vT_sb)):
                tsl = slice(0, QCH)
                pj = ps.tile([P, QCH], F32, tag="aux", bufs=2, name="pj")
                for ci in range(NCH):
                    nc.tensor.matmul(
                        pj[:], wsb[:, ci, :], xT_sb[:, ci, tsl],
                        start=(ci == 0), stop=(ci == NCH - 1),
                    )
                nc.vector.tensor_copy(dst[:, tsl], pj[:])
            for t32 in range(4):
                make_vtrans_item(t32)()

            # seed filler with the next projections
            filler.extend((1, it) for it in proj_items(1))
            filler.extend((2, it) for it in proj_items(2))

            # ---- main loop over (b, jq) ----
            # slot s = b*NQC + jq uses projection chunk tch=s (tokens
            # [s*512,(s+1)*512)); tch s+2 is appended to the filler at
            # slot s.  O-proj chunks enter the filler only once their
            # gather can have physically completed on the serial
            # collective stream (init ends ~72us; each op 10-30us).
            SLOTS = [(b, jq) for b in range(B) for jq in range(NQC)]
            AG_AT = {3: 0, 5: 1, 6: 2}      # slot end -> gather idx
            PO_START = {7: (2, 3, 4, 5)}    # slot start -> out chunks
            PO_MID = {6: (0, 1), 7: (6,)}   # after h0 -> out chunks

            def append_po(chunks):
                for c in chunks:
                    yf = yf_load(c)
                    filler.extend((99, it) for it in make_po_items(c, yf))

            for s, (b, jq) in enumerate(SLOTS):
                if s >= 1:
                    drain_tch(s)  # projections this slot reads
                    if s + 2 < len(SLOTS):
                        filler.extend((s + 2, it)
                                      for it in proj_items(s + 2))
                append_po(PO_START.get(s, ()))
                # pace filler so items due next slot drain this slot;
                # surplus beyond that drains at >=2/pair
                npairs = 4 * (jq + 1)
                due = sum(1 for k, _ in filler if k <= s + 1)
                per_pair = max(2, min(6, -(-due // npairs)))
                for h in range(HL):
                    yt, den = attn_compute(b, jq, h, per_pair=per_pair)
                    attn_evict(b, jq, h, yt, den)
                    if h == 0:
                        append_po(PO_MID.get(s, ()))
                if s in AG_AT:
                    ag_fire(AG_AT[s])
            ag_fire(3)

            # ---- tail: drain leftovers, keep PE warm, last O-proj ----
            drain_filler()
            warm(10)
            yf7 = yf_load(7)
            for it in make_po_items(7, yf7, last=True):
                it()

    nc.finalize()
    return nc


_GRAPH = None


def _get_graph():
    global _GRAPH
    if _GRAPH is None:
        _GRAPH = build_graph()
    return _GRAPH


def prepare_in_maps(x, Wq, Wk, Wv, Wo):
    x = np.asarray(x, np.float32)
    Wq = np.asarray(Wq, np.float32)
    Wk = np.asarray(Wk, np.float32)
    Wv = np.asarray(Wv, np.float32)
    Wo = np.asarray(Wo, np.float32)

    bf = ml_dtypes.bfloat16
    xTh = np.ascontiguousarray(x.reshape(TT, C).T).astype(bf)
    in_maps = []
    for r in range(N_CORES):
        sl = slice(r * DL, (r + 1) * DL)
        # pack the 4 transposed weight shards into the SBUF layout
        # [p, w, ci, m] where the shard row index is c = ci*128 + p
        wall = np.empty((P, 4, NCH, DL), np.float32)
        for w, W in enumerate((Wq, Wk, Wv, Wo)):
            wall[:, w] = W[sl].T.reshape(NCH, P, DL).transpose(1, 0, 2)
        in_maps.append({
            "xT": xTh,
            "wall": np.ascontiguousarray(
                wall.reshape(P, 4 * NCH * DL)).astype(bf),
        })
    return in_maps


def assemble_output(results):
    outT = np.concatenate(
        [np.asarray(results[r]["out"], np.float32) for r in range(N_CORES)],
        axis=0,
    )  # [C, TT]
    return np.ascontiguousarray(outT.T).reshape(B, T, C)


def kernel(x, Wq, Wk, Wv, Wo):
    nc = _get_graph()
    in_maps = prepare_in_maps(x, Wq, Wk, Wv, Wo)
    res = run_bass_kernel_spmd(nc, in_maps, core_ids=list(range(N_CORES)))
    return assemble_output(res.results)


# revision 14
# speedup vs baseline: 1.1806x; 1.0272x over previous
"""Causal multi-head attention block (B=2, T=2048, C=1024, H=16) on 8 TRN2
NeuronCores.

Sharding (v4): 2D batch x head-group.  Core r = 4*g + i (g = batch, i =
group rank) owns heads [4i, 4i+4) of batch g, i.e. feature rows
[256i, 256i+256) of Wq/Wk/Wv, and output rows [256i, 256i+256) of the
final projection for batch g.  The y AllGather then runs inside each
4-core group (output 4 MB instead of 8 MB, and the two groups' gathers
run concurrently on separate dies) and each core only loads its own
batch's activations (4 MB instead of 8 MB).

On-device everything is computed feature-major (transposed) so the
TensorEngine contraction axis sits on SBUF partitions, and the softmax
denominator comes free via a ones-column appended to V:

  qT/kT/vT [128, ft, 2048] = W_shard @ x^T   (2 feature tiles of 128 =
                                              2 head-pairs)
  ST tile [128k, 512q] = kT_rows.T @ qT_rows (contract d=64)
  causal mask: matmul(ident, mneg) adds a -1e9 strictly-lower triangle
      into the St PSUM group on diagonal blocks
  PT = exp(ST / sqrt(d))     (logits ~N(0,1); no max subtraction)
  yT [65, 512] += [v | 1].T @ PT             (row 64 = denominator)
  y  = yT[0:64] * partition_broadcast(recip(yT[64]))

Performance structure (from perfetto/HAM analysis of v1-v3):
- The exp stream on ACT (~1.15us per 1024-col ACTIVATE, 80 of them) is
  the kernel's clock; the PE must stay dense to hold HAM at K=8/8.
  All projections beyond the very first q/k/v tile and the whole
  O-projection are split into single-matmul filler items popped between
  the exp-gated attention pairs.
- The scalar (ACT) queue carries only the exps plus 3 batched xT input
  DMA issues at t=0.  Large dma_starts fan across all 16 SDMA engines.
- The collective stream is serial per core and unavailable for the
  first ~60us (comm-stack init); each op also starts ~15-30us after its
  trigger.  Gathers are therefore asymmetric -- {jq0+jq1}, {jq2},
  {jq3} -- and O-proj chunks only enter the filler once their gather
  can physically have completed; the last small gather bounds the tail.
- PSUM tags: st 2x2 banks, yt 2x1, aux 2x1 (proj/O-proj/transpose/warm).

Inputs are bf16 (host-side cast); accumulation is f32 in PSUM; the
output shard is written bf16 and upcast to f32 on the host.
"""

import numpy as np
import ml_dtypes

import concourse.bacc as bacc
import concourse.mybir as mybir
import concourse.tile as tile
from concourse.bass_utils import run_bass_kernel_spmd
from concourse.masks import make_identity

N_CORES = 8
B, T, C, H = 2, 2048, 1024, 16
D = 64                # head dim
GW = 4                # group width (cores per batch group)
HL = H // GW          # heads per core = 4
DL = HL * D           # local feature dim = 256
NFT = DL // 128       # feature tiles per core = 2
TL = T                # local tokens = one batch = 2048
P = 128
NCH = C // P          # 8 contraction chunks
QCH = 512             # q-chunk (moving free dim)
NQC = TL // QCH       # 4 q-chunks
NKT = TL // P         # 16 k-tiles
SCALE = 1.0 / np.sqrt(D)

BF = mybir.dt.bfloat16
F32 = mybir.dt.float32
AF = mybir.ActivationFunctionType

REPLICA_GROUPS = [[0, 1, 2, 3], [4, 5, 6, 7]]


def build_graph():
    nc = bacc.Bacc("TRN2", target_bir_lowering=False, debug=False)

    xT = nc.dram_tensor("xT", [C, TL], BF, kind="ExternalInput")
    # 4 weight shards pre-packed host-side into SBUF layout [p, w, ci, m]
    wall = nc.dram_tensor("wall", [P, 4 * NCH * DL], BF,
                          kind="ExternalInput")
    out = nc.dram_tensor("out", [DL, TL], BF, kind="ExternalOutput")

    with tile.TileContext(nc) as tc:
        with (
            tc.tile_pool(name="sb", bufs=1) as sb,
            tc.tile_pool(name="ps", bufs=1, space="PSUM") as ps,
            tc.tile_pool(name="dram", bufs=1, space="DRAM") as dram,
        ):
            # ---- input loads ----
            w_sb = sb.tile([P, 4 * NCH * DL], BF, name="w_sb")
            nc.sync.dma_start(w_sb[:, :], wall[:, :])
            w4 = w_sb[:].rearrange("p (w a m) -> p w a m", w=4, a=NCH)

            xT_sb = sb.tile([P, NCH, TL], BF, name="xT_sb")
            xTr = xT[:, :].rearrange("(a p) t -> p a t", p=P)
            for s0, s1 in ((0, QCH), (QCH, 2 * QCH), (2 * QCH, TL)):
                nc.scalar.dma_start(xT_sb[:, :, s0:s1], xTr[:, :, s0:s1])

            ident = sb.tile([P, P], BF, name="ident")
            make_identity(nc, ident)
            # strictly-lower-triangular -1e9 (k > q)
            mneg = sb.tile([P, P], BF, name="mneg")
            nc.gpsimd.memset(mneg[:], 0.0)
            nc.gpsimd.affine_select(
                out=mneg[:], in_=mneg[:],
                compare_op=mybir.AluOpType.is_ge,
                fill=-1e9, base=0, channel_multiplier=-1, pattern=[[1, P]],
            )
            wsrc = sb.tile([P, QCH], BF, name="wsrc")
            nc.vector.memset(wsrc[:], 0.5)

            def warm(n):
                for _ in range(n):
                    wdst = ps.tile([P, QCH], F32, tag="aux", bufs=2,
                                   name="wdst")
                    nc.tensor.matmul(wdst[:], ident[:], wsrc[:],
                                     start=True, stop=True)

            warm(6)

            qT_sb = sb.tile([P, NFT, TL], BF, name="qT_sb")
            kT_sb = sb.tile([P, NFT, TL], BF, name="kT_sb")
            vT_sb = sb.tile([P, NFT, TL], BF, name="vT_sb")
            # v natural layout per 128-token tile:
            # [h0(64) | 1 | h1 | 1 | h2 | 1 | h3 | 1] -> 260 columns
            v_sb = sb.tile([P, NKT, HL * (D + 1)], BF, name="v_sb")
            nc.gpsimd.memset(v_sb[:], 1.0)

            # ---- AllGather buffers ----
            # A: jq0+jq1 [256, 1024], B: jq2 [256, 512], C: jq3 [256,512]
            AG_NJQ = (2, 1, 1)
            ag_in = [dram.tile([DL, n * QCH], BF, name=f"ag_in{g}")
                     for g, n in enumerate(AG_NJQ)]
            ytf = [dram.tile([C, n * QCH], BF, name=f"ytf{g}")
                   for g, n in enumerate(AG_NJQ)]
            AG_OF = {0: (0, 0), 1: (0, 1), 2: (1, 0), 3: (2, 0)}

            # ---- work items ----
            def make_proj_items(tch, ft, w, dst):
                tsl = slice(tch * QCH, (tch + 1) * QCH)
                msl = slice(ft * P, (ft + 1) * P)
                state = {}
                items = []
                for ci in range(NCH):
                    def mm(ci=ci):
                        if ci == 0:
                            state['pj'] = ps.tile([P, QCH], F32, tag="aux",
                                                  bufs=2, name="pj")
                        nc.tensor.matmul(
                            state['pj'][:], w4[:, w, ci, msl],
                            xT_sb[:, ci, tsl],
                            start=(ci == 0), stop=(ci == NCH - 1),
                        )
                        if ci == NCH - 1:
                            nc.vector.tensor_copy(dst[:, ft, tsl],
                                                  state['pj'][:])
                    items.append(mm)
                return items

            def make_vtrans_item(ft, t32):
                def it():
                    tr = ps.tile([P, P], BF, tag="aux", bufs=2, name="tr")
                    nc.tensor.transpose(
                        tr[:], vT_sb[:, ft, t32 * P:(t32 + 1) * P], ident[:]
                    )
                    out_ap = v_sb[:, t32, :].rearrange(
                        "p (h x) -> p h x", h=HL
                    )[:, 2 * ft:2 * ft + 2, 0:D]
                    in_ap = tr[:].rearrange("p (h x) -> p h x", h=2)
                    nc.vector.tensor_copy(out_ap, in_ap)
                return it

            def proj_items(tch, ft):
                items = []
                for w, dst in ((0, qT_sb), (1, kT_sb), (2, vT_sb)):
                    items.extend(make_proj_items(tch, ft, w, dst))
                for t32 in range(tch * 4, tch * 4 + 4):
                    items.append(make_vtrans_item(ft, t32))
                return items

            def yf_load(c):
                g, off = AG_OF[c]
                yf = sb.tile([P, NCH, QCH], BF, tag="yf", bufs=2, name="yf")
                src = ytf[g][:, off * QCH:(off + 1) * QCH].rearrange(
                    "(a p) t -> p a t", p=P)
                nc.sync.dma_start(yf[:, :, :], src)
                return yf

            def make_po_items(c, yf, last=False):
                c0 = c * QCH
                items = []
                for mb in range(NFT):
                    state = {}
                    for ci in range(NCH):
                        def mm(ci=ci, mb=mb, state=state):
                            if ci == 0:
                                state['po'] = ps.tile(
                                    [P, QCH], F32, tag="aux", bufs=2,
                                    name="po")
                            nc.tensor.matmul(
                                state['po'][:],
                                w4[:, 3, ci, mb * P:(mb + 1) * P],
                                yf[:, ci, :],
                                start=(ci == 0), stop=(ci == NCH - 1),
                            )
                            if ci == NCH - 1:
                                ob = sb.tile([P, QCH], BF, tag="ob",
                                             bufs=2, name="ob")
                                nc.vector.tensor_copy(ob[:], state['po'][:])
                                nsp = 4 if last else 1
                                w_ = QCH // nsp
                                for sp in range(nsp):
                                    nc.sync.dma_start(
                                        out[mb * P:(mb + 1) * P,
                                            c0 + sp * w_:c0 + (sp + 1) * w_],
                                        ob[:, sp * w_:(sp + 1) * w_],
                                    )
                        items.append(mm)
                return items

            # ---- filler machinery ----
            # FIFO of (key, fn); key = 2*tch + ft for projection items
            # (deadline: key 2s before slot s, 2s+1 before its head 2),
            # 99 for O-proj items (no deadline).
            filler = []

            def pop_filler(n):
                for _ in range(min(n, len(filler))):
                    filler.pop(0)[1]()

            def drain_key(kmax):
                while any(k <= kmax for k, _ in filler):
                    filler.pop(0)[1]()

            def drain_filler():
                while filler:
                    filler.pop(0)[1]()

            # ---- attention ----
            def attn_compute(jq, h, per_pair=2):
                th, hr = h // 2, h % 2
                rsl = slice(hr * D, (hr + 1) * D)
                q0 = jq * QCH
                yt = ps.tile([D + 1, QCH], F32, tag="yt", bufs=2, name="yt")
                nkt = 4 * jq + 4
                for pr in range(nkt // 2):
                    st = ps.tile([P, 2 * QCH], F32, tag="st", bufs=2,
                                 name="st")
                    pt = sb.tile([P, 2 * QCH], BF, tag="pt", bufs=4,
                                 name="pt")
                    for half in range(2):
                        kt = 2 * pr + half
                        k0 = kt * P
                        i = kt - 4 * jq
                        qv = max(i, 0) * P
                        ssl = slice(half * QCH + qv, (half + 1) * QCH)
                        nc.tensor.matmul(
                            st[:, ssl],
                            kT_sb[rsl, th, k0:k0 + P],
                            qT_sb[rsl, th, q0 + qv:q0 + QCH],
                            start=True, stop=(i < 0),
                        )
                        if i >= 0:
                            nc.tensor.matmul(
                                st[:, half * QCH + qv:half * QCH + qv + P],
                                ident[:], mneg[:],
                                start=False, stop=True,
                            )
                    qv0 = max(2 * pr - 4 * jq, 0) * P
                    nc.scalar.activation(
                        pt[:, qv0:], st[:, qv0:], AF.Exp, scale=float(SCALE)
                    )
                    # PE filler between the exp issue and the AV matmuls
                    pop_filler(per_pair)
                    for half in range(2):
                        kt = 2 * pr + half
                        qv = max(kt - 4 * jq, 0) * P
                        nc.tensor.matmul(
                            yt[:, qv:QCH],
                            v_sb[:, kt, h * (D + 1):(h + 1) * (D + 1)],
                            pt[:, half * QCH + qv:(half + 1) * QCH],
                            start=(kt == 0), stop=(kt == nkt - 1),
                        )
                den = sb.tile([1, QCH], F32, tag="den", bufs=4, name="den")
                nc.vector.tensor_copy(den[:], yt[D:D + 1, :])
                return yt, den

            def attn_evict(jq, h, yt, den):
                bc = sb.tile([D, QCH], F32, tag="bc", bufs=3, name="bc")
                nc.gpsimd.partition_broadcast(bc[:], den[:])
                rcp = sb.tile([D, QCH], F32, tag="rcp", bufs=3, name="rcp")
                scr = sb.tile([D, QCH], F32, tag="scr", bufs=3, name="scr")
                nc.vector.reciprocal_approx_accurate(
                    rcp[:], bc[:], scratch=scr[:]
                )
                yn = sb.tile([D, QCH], BF, tag="yn", bufs=4, name="yn")
                nc.vector.tensor_mul(yn[:], yt[0:D, :], rcp[:])
                g, off = AG_OF[jq]
                nc.gpsimd.dma_start(
                    ag_in[g][h * D:(h + 1) * D,
                             off * QCH:(off + 1) * QCH],
                    yn[:, :],
                )

            def ag_fire(g):
                nc.gpsimd.collective_compute(
                    "AllGather", mybir.AluOpType.bypass,
                    replica_groups=REPLICA_GROUPS,
                    ins=[ag_in[g][:]], outs=[ytf[g][:]],
                )

            # ---- prologue: tch0 feature-tile 0 only ----
            for w, dst in ((0, qT_sb), (1, kT_sb), (2, vT_sb)):
                pj = ps.tile([P, QCH], F32, tag="aux", bufs=2, name="pj")
                for ci in range(NCH):
                    nc.tensor.matmul(
                        pj[:], w4[:, w, ci, 0:P], xT_sb[:, ci, 0:QCH],
                        start=(ci == 0), stop=(ci == NCH - 1),
                    )
                nc.vector.tensor_copy(dst[:, 0, 0:QCH], pj[:])
            for t32 in range(4):
                make_vtrans_item(0, t32)()

            filler.extend((1, it) for it in proj_items(0, 1))
            filler.extend((2, it) for it in proj_items(1, 0))
            filler.extend((3, it) for it in proj_items(1, 1))

            # ---- main loop over jq slots ----
            AG_AT = {1: 0, 2: 1}          # gathers fired at slot end
            # O-proj chunks appended after head hx of slot s (gather
            # must have physically completed by then)
            PO_MID = {(3, 0): (0,), (3, 1): (1,), (3, 2): (2,)}

            def append_po(chunks, last=False):
                for c in chunks:
                    yf = yf_load(c)
                    filler.extend(
                        (99, it) for it in make_po_items(c, yf, last=last))

            for s in range(NQC):
                if s >= 1:
                    drain_key(2 * s)
                if s + 2 <= NQC - 1:
                    # tch s+2 projections join the filler at slot s
                    filler.extend((2 * (s + 2), it)
                                  for it in proj_items(s + 2, 0))
                    filler.extend((2 * (s + 2) + 1, it)
                                  for it in proj_items(s + 2, 1))
                npairs = 8 * (s + 1)
                due = sum(1 for k, _ in filler if k <= 2 * s + 3)
                per_pair = max(2, min(6, -(-due // npairs)))
                for h in range(HL):
                    if h == 2:
                        drain_key(2 * s + 1)
                    yt, den = attn_compute(s, h, per_pair=per_pair)
                    attn_evict(s, h, yt, den)
                    append_po(PO_MID.get((s, h), ()))
                if s in AG_AT:
                    ag_fire(AG_AT[s])
            ag_fire(2)

            # ---- tail ----
            drain_filler()
            warm(8)
            append_po((3,), last=True)
            drain_filler()

    nc.finalize()
    return nc


_GRAPH = None


def _get_graph():
    global _GRAPH
    if _GRAPH is None:
        _GRAPH = build_graph()
    return _GRAPH


def prepare_in_maps(x, Wq, Wk, Wv, Wo):
    x = np.asarray(x, np.float32)
    Wq = np.asarray(Wq, np.float32)
    Wk = np.asarray(Wk, np.float32)
    Wv = np.asarray(Wv, np.float32)
    Wo = np.asarray(Wo, np.float32)

    bf = ml_dtypes.bfloat16
    xTh = [np.ascontiguousarray(x[g].T).astype(bf) for g in range(B)]
    in_maps = []
    for r in range(N_CORES):
        g, i = r // GW, r % GW
        sl = slice(i * DL, (i + 1) * DL)
        wall = np.empty((P, 4, NCH, DL), np.float32)
        for w, W in enumerate((Wq, Wk, Wv, Wo)):
            wall[:, w] = W[sl].T.reshape(NCH, P, DL).transpose(1, 0, 2)
        in_maps.append({
            "xT": xTh[g],
            "wall": np.ascontiguousarray(
                wall.reshape(P, 4 * NCH * DL)).astype(bf),
        })
    return in_maps


def assemble_output(results):
    outs = []
    for g in range(B):
        outT = np.concatenate(
            [np.asarray(results[GW * g + i]["out"], np.float32)
             for i in range(GW)], axis=0)  # [C, TL]
        outs.append(outT.T)
    return np.ascontiguousarray(np.stack(outs))  # [B, T, C]


def kernel(x, Wq, Wk, Wv, Wo):
    nc = _get_graph()
    in_maps = prepare_in_maps(x, Wq, Wk, Wv, Wo)
    res = run_bass_kernel_spmd(nc, in_maps, core_ids=list(range(N_CORES)))
    return assemble_output(res.results)


# revision 17
# speedup vs baseline: 1.3739x; 1.1637x over previous
"""Causal multi-head attention block (B=2, T=2048, C=1024, H=16) on 8 TRN2
NeuronCores.

Sharding (v4): 2D batch x head-group.  Core r = 4*g + i (g = batch, i =
group rank) owns heads [4i, 4i+4) of batch g, i.e. feature rows
[256i, 256i+256) of Wq/Wk/Wv, and output rows [256i, 256i+256) of the
final projection for batch g.  The y AllGather then runs inside each
4-core group (output 4 MB instead of 8 MB, and the two groups' gathers
run concurrently on separate dies) and each core only loads its own
batch's activations (4 MB instead of 8 MB).

On-device everything is computed feature-major (transposed) so the
TensorEngine contraction axis sits on SBUF partitions, and the softmax
denominator comes free via a ones-column appended to V:

  qT/kT/vT [128, ft, 2048] = W_shard @ x^T   (2 feature tiles of 128 =
                                              2 head-pairs)
  ST tile [128k, 512q] = kT_rows.T @ qT_rows (contract d=64)
  causal mask: matmul(ident, mneg) adds a -1e9 strictly-lower triangle
      into the St PSUM group on diagonal blocks
  PT = exp(ST / sqrt(d))     (logits ~N(0,1); no max subtraction)
  yT [65, 512] += [v | 1].T @ PT             (row 64 = denominator)
  y  = yT[0:64] * partition_broadcast(recip(yT[64]))

Performance structure (from perfetto/HAM analysis of v1-v3):
- The exp stream on ACT (~1.15us per 1024-col ACTIVATE, 80 of them) is
  the kernel's clock; the PE must stay dense to hold HAM at K=8/8.
  All projections beyond the very first q/k/v tile and the whole
  O-projection are split into single-matmul filler items popped between
  the exp-gated attention pairs.
- The scalar (ACT) queue carries only the exps plus 3 batched xT input
  DMA issues at t=0.  Large dma_starts fan across all 16 SDMA engines.
- The collective stream is serial per core and unavailable for the
  first ~60us (comm-stack init); each op also starts ~15-30us after its
  trigger.  Gathers are therefore asymmetric -- {jq0+jq1}, {jq2},
  {jq3} -- and O-proj chunks only enter the filler once their gather
  can physically have completed; the last small gather bounds the tail.
- PSUM tags: st 2x2 banks, yt 2x1, aux 2x1 (proj/O-proj/transpose/warm).

Inputs are bf16 (host-side cast); accumulation is f32 in PSUM; the
output shard is written bf16 and upcast to f32 on the host.
"""

import numpy as np
import ml_dtypes

import concourse.bacc as bacc
import concourse.mybir as mybir
import concourse.tile as tile
from concourse.bass_utils import run_bass_kernel_spmd
from concourse.masks import make_identity

N_CORES = 8
B, T, C, H = 2, 2048, 1024, 16
D = 64                # head dim
GW = 4                # group width (cores per batch group)
HL = H // GW          # heads per core = 4
DL = HL * D           # local feature dim = 256
NFT = DL // 128       # feature tiles per core = 2
TL = T                # local tokens = one batch = 2048
P = 128
NCH = C // P          # 8 contraction chunks
QCH = 512             # q-chunk (moving free dim)
NQC = TL // QCH       # 4 q-chunks
NKT = TL // P         # 16 k-tiles
SCALE = 1.0 / np.sqrt(D)

BF = mybir.dt.bfloat16
F32 = mybir.dt.float32
AF = mybir.ActivationFunctionType

REPLICA_GROUPS = [[0, 1, 2, 3], [4, 5, 6, 7]]


def build_graph():
    nc = bacc.Bacc("TRN2", target_bir_lowering=False, debug=False)

    xT = nc.dram_tensor("xT", [C, TL], BF, kind="ExternalInput")
    # 4 weight shards pre-packed host-side into SBUF layout [p, w, ci, m]
    wall = nc.dram_tensor("wall", [P, 4 * NCH * DL], BF,
                          kind="ExternalInput")
    out = nc.dram_tensor("out", [DL, TL], BF, kind="ExternalOutput")

    with tile.TileContext(nc) as tc:
        with (
            tc.tile_pool(name="sb", bufs=1) as sb,
            tc.tile_pool(name="ps", bufs=1, space="PSUM") as ps,
            tc.tile_pool(name="dram", bufs=1, space="DRAM") as dram,
        ):
            # ---- input loads ----
            w_sb = sb.tile([P, 4 * NCH * DL], BF, name="w_sb")
            WCOLS = 4 * NCH * DL
            for pc in range(8):
                csl = slice(pc * (WCOLS // 8), (pc + 1) * (WCOLS // 8))
                nc.sync.dma_start(w_sb[:, csl], wall[:, csl])
            w4 = w_sb[:].rearrange("p (w a m) -> p w a m", w=4, a=NCH)

            xT_sb = sb.tile([P, NCH, TL], BF, name="xT_sb")
            xTr = xT[:, :].rearrange("(a p) t -> p a t", p=P)
            for s0, s1 in ((0, QCH), (QCH, 2 * QCH), (2 * QCH, TL)):
                nc.scalar.dma_start(xT_sb[:, :, s0:s1], xTr[:, :, s0:s1])

            ident = sb.tile([P, P], BF, name="ident")
            make_identity(nc, ident)
            # strictly-lower-triangular -1e9 (k > q)
            mneg = sb.tile([P, P], BF, name="mneg")
            nc.gpsimd.memset(mneg[:], 0.0)
            nc.gpsimd.affine_select(
                out=mneg[:], in_=mneg[:],
                compare_op=mybir.AluOpType.is_ge,
                fill=-1e9, base=0, channel_multiplier=-1, pattern=[[1, P]],
            )
            wsrc = sb.tile([P, QCH], BF, name="wsrc")
            nc.vector.memset(wsrc[:], 0.5)

            def warm(n):
                for _ in range(n):
                    wdst = ps.tile([P, QCH], F32, tag="aux", bufs=2,
                                   name="wdst")
                    nc.tensor.matmul(wdst[:], ident[:], wsrc[:],
                                     start=True, stop=True)

            warm(6)

            qT_sb = sb.tile([P, NFT, TL], BF, name="qT_sb")
            kT_sb = sb.tile([P, NFT, TL], BF, name="kT_sb")
            vT_sb = sb.tile([P, NFT, TL], BF, name="vT_sb")
            # v natural layout per 128-token tile:
            # [h0(64) | 1 | h1 | 1 | h2 | 1 | h3 | 1] -> 260 columns
            v_sb = sb.tile([P, NKT, HL * (D + 1)], BF, name="v_sb")
            nc.gpsimd.memset(v_sb[:], 1.0)

            # ---- AllGather buffers: one gather per jq slot ----
            ag_in = [dram.tile([DL, QCH], BF, name=f"ag_in{g}")
                     for g in range(NQC)]
            ytf = [dram.tile([C, QCH], BF, name=f"ytf{g}")
                   for g in range(NQC)]
            AG_OF = {jq: (jq, 0) for jq in range(NQC)}

            # ---- work items ----
            def make_proj_items(tch, ft, w, dst):
                tsl = slice(tch * QCH, (tch + 1) * QCH)
                msl = slice(ft * P, (ft + 1) * P)
                state = {}
                items = []
                for ci in range(NCH):
                    def mm(ci=ci):
                        if ci == 0:
                            state['pj'] = ps.tile([P, QCH], F32, tag="aux",
                                                  bufs=2, name="pj")
                        nc.tensor.matmul(
                            state['pj'][:], w4[:, w, ci, msl],
                            xT_sb[:, ci, tsl],
                            start=(ci == 0), stop=(ci == NCH - 1),
                        )
                        if ci == NCH - 1:
                            nc.vector.tensor_copy(dst[:, ft, tsl],
                                                  state['pj'][:])
                    items.append(mm)
                return items

            def make_vtrans_item(ft, t32):
                def it():
                    tr = ps.tile([P, P], BF, tag="aux", bufs=2, name="tr")
                    nc.tensor.transpose(
                        tr[:], vT_sb[:, ft, t32 * P:(t32 + 1) * P], ident[:]
                    )
                    out_ap = v_sb[:, t32, :].rearrange(
                        "p (h x) -> p h x", h=HL
                    )[:, 2 * ft:2 * ft + 2, 0:D]
                    in_ap = tr[:].rearrange("p (h x) -> p h x", h=2)
                    nc.vector.tensor_copy(out_ap, in_ap)
                return it

            def proj_items(tch, ft):
                items = []
                for w, dst in ((0, qT_sb), (1, kT_sb), (2, vT_sb)):
                    items.extend(make_proj_items(tch, ft, w, dst))
                for t32 in range(tch * 4, tch * 4 + 4):
                    items.append(make_vtrans_item(ft, t32))
                return items

            def yf_load(c):
                g, off = AG_OF[c]
                yf = sb.tile([P, NCH, QCH], BF, tag="yf", bufs=2, name="yf")
                src = ytf[g][:, off * QCH:(off + 1) * QCH].rearrange(
                    "(a p) t -> p a t", p=P)
                nc.sync.dma_start(yf[:, :, :], src)
                return yf

            def make_po_items(c, yf, last=False):
                c0 = c * QCH
                items = []
                for mb in range(NFT):
                    state = {}
                    for ci in range(NCH):
                        def mm(ci=ci, mb=mb, state=state):
                            if ci == 0:
                                state['po'] = ps.tile(
                                    [P, QCH], F32, tag="aux", bufs=2,
                                    name="po")
                            nc.tensor.matmul(
                                state['po'][:],
                                w4[:, 3, ci, mb * P:(mb + 1) * P],
                                yf[:, ci, :],
                                start=(ci == 0), stop=(ci == NCH - 1),
                            )
                            if ci == NCH - 1:
                                ob = sb.tile([P, QCH], BF, tag="ob",
                                             bufs=2, name="ob")
                                nc.vector.tensor_copy(ob[:], state['po'][:])
                                nsp = 4 if last else 1
                                w_ = QCH // nsp
                                for sp in range(nsp):
                                    nc.sync.dma_start(
                                        out[mb * P:(mb + 1) * P,
                                            c0 + sp * w_:c0 + (sp + 1) * w_],
                                        ob[:, sp * w_:(sp + 1) * w_],
                                    )
                        items.append(mm)
                return items

            # ---- filler machinery ----
            # FIFO of (key, fn); key = 2*tch + ft for projection items
            # (deadline: key 2s before slot s, 2s+1 before its head 2),
            # 99 for O-proj items (no deadline).
            filler = []

            def pop_filler(n):
                for _ in range(min(n, len(filler))):
                    filler.pop(0)[1]()

            def drain_key(kmax):
                while any(k <= kmax for k, _ in filler):
                    filler.pop(0)[1]()

            def drain_filler():
                while filler:
                    filler.pop(0)[1]()

            # ---- attention ----
            def attn_compute(jq, h, per_pair=2):
                th, hr = h // 2, h % 2
                rsl = slice(hr * D, (hr + 1) * D)
                q0 = jq * QCH
                yt = ps.tile([D + 1, QCH], F32, tag="yt", bufs=2, name="yt")
                nkt = 4 * jq + 4
                for pr in range(nkt // 2):
                    st = ps.tile([P, 2 * QCH], F32, tag="st", bufs=2,
                                 name="st")
                    pt = sb.tile([P, 2 * QCH], BF, tag="pt", bufs=4,
                                 name="pt")
                    for half in range(2):
                        kt = 2 * pr + half
                        k0 = kt * P
                        i = kt - 4 * jq
                        qv = max(i, 0) * P
                        ssl = slice(half * QCH + qv, (half + 1) * QCH)
                        nc.tensor.matmul(
                            st[:, ssl],
                            kT_sb[rsl, th, k0:k0 + P],
                            qT_sb[rsl, th, q0 + qv:q0 + QCH],
                            start=True, stop=(i < 0),
                        )
                        if i >= 0:
                            nc.tensor.matmul(
                                st[:, half * QCH + qv:half * QCH + qv + P],
                                ident[:], mneg[:],
                                start=False, stop=True,
                            )
                    qv0 = max(2 * pr - 4 * jq, 0) * P
                    nc.scalar.activation(
                        pt[:, qv0:], st[:, qv0:], AF.Exp, scale=float(SCALE)
                    )
                    # PE filler between the exp issue and the AV matmuls
                    pop_filler(per_pair)
                    for half in range(2):
                        kt = 2 * pr + half
                        qv = max(kt - 4 * jq, 0) * P
                        nc.tensor.matmul(
                            yt[:, qv:QCH],
                            v_sb[:, kt, h * (D + 1):(h + 1) * (D + 1)],
                            pt[:, half * QCH + qv:(half + 1) * QCH],
                            start=(kt == 0), stop=(kt == nkt - 1),
                        )
                den = sb.tile([1, QCH], F32, tag="den", bufs=4, name="den")
                nc.vector.tensor_copy(den[:], yt[D:D + 1, :])
                return yt, den

            def attn_evict(jq, h, yt, den):
                bc = sb.tile([D, QCH], F32, tag="bc", bufs=3, name="bc")
                nc.gpsimd.partition_broadcast(bc[:], den[:])
                rcp = sb.tile([D, QCH], F32, tag="rcp", bufs=3, name="rcp")
                scr = sb.tile([D, QCH], F32, tag="scr", bufs=3, name="scr")
                nc.vector.reciprocal_approx_accurate(
                    rcp[:], bc[:], scratch=scr[:]
                )
                yn = sb.tile([D, QCH], BF, tag="yn", bufs=4, name="yn")
                nc.vector.tensor_mul(yn[:], yt[0:D, :], rcp[:])
                g, off = AG_OF[jq]
                nc.gpsimd.dma_start(
                    ag_in[g][h * D:(h + 1) * D,
                             off * QCH:(off + 1) * QCH],
                    yn[:, :],
                )

            def ag_fire(g):
                nc.gpsimd.collective_compute(
                    "AllGather", mybir.AluOpType.bypass,
                    replica_groups=REPLICA_GROUPS,
                    ins=[ag_in[g][:]], outs=[ytf[g][:]],
                )

            # ---- prologue: tch0 feature-tile 0 only ----
            for w, dst in ((0, qT_sb), (1, kT_sb), (2, vT_sb)):
                pj = ps.tile([P, QCH], F32, tag="aux", bufs=2, name="pj")
                for ci in range(NCH):
                    nc.tensor.matmul(
                        pj[:], w4[:, w, ci, 0:P], xT_sb[:, ci, 0:QCH],
                        start=(ci == 0), stop=(ci == NCH - 1),
                    )
                nc.vector.tensor_copy(dst[:, 0, 0:QCH], pj[:])
            for t32 in range(4):
                make_vtrans_item(0, t32)()

            filler.extend((1, it) for it in proj_items(0, 1))
            filler.extend((2, it) for it in proj_items(1, 0))
            filler.extend((3, it) for it in proj_items(1, 1))

            # ---- main loop over jq slots ----
            # O-proj chunk c enters the filler only once gather c has
            # physically completed (fired at end of slot c; ~30us wall)
            PO_START = {2: (0,), 3: (1,)}
            PO_MID = {(3, 2): (2,)}

            def append_po(chunks, last=False):
                for c in chunks:
                    yf = yf_load(c)
                    filler.extend(
                        (99, it) for it in make_po_items(c, yf, last=last))

            for s in range(NQC):
                if s >= 1:
                    drain_key(2 * s)
                if s + 2 <= NQC - 1:
                    # tch s+2 projections join the filler at slot s
                    filler.extend((2 * (s + 2), it)
                                  for it in proj_items(s + 2, 0))
                    filler.extend((2 * (s + 2) + 1, it)
                                  for it in proj_items(s + 2, 1))
                append_po(PO_START.get(s, ()))
                npairs = 8 * (s + 1)
                due = sum(1 for k, _ in filler if k <= 2 * s + 2)
                per_pair = max(2, min(6, -(-due // npairs)))
                for h in range(HL):
                    if h == 2:
                        drain_key(2 * s + 1)
                    yt, den = attn_compute(s, h, per_pair=per_pair)
                    attn_evict(s, h, yt, den)
                    append_po(PO_MID.get((s, h), ()))
                ag_fire(s)

            # ---- tail ----
            drain_filler()
            warm(8)
            append_po((3,), last=True)
            drain_filler()

    nc.finalize()
    return nc


_GRAPH = None


def _get_graph():
    global _GRAPH
    if _GRAPH is None:
        _GRAPH = build_graph()
    return _GRAPH


def prepare_in_maps(x, Wq, Wk, Wv, Wo):
    x = np.asarray(x, np.float32)
    Wq = np.asarray(Wq, np.float32)
    Wk = np.asarray(Wk, np.float32)
    Wv = np.asarray(Wv, np.float32)
    Wo = np.asarray(Wo, np.float32)

    bf = ml_dtypes.bfloat16
    xTh = [np.ascontiguousarray(x[g].T).astype(bf) for g in range(B)]
    in_maps = []
    for r in range(N_CORES):
        g, i = r // GW, r % GW
        sl = slice(i * DL, (i + 1) * DL)
        wall = np.empty((P, 4, NCH, DL), np.float32)
        for w, W in enumerate((Wq, Wk, Wv, Wo)):
            wall[:, w] = W[sl].T.reshape(NCH, P, DL).transpose(1, 0, 2)
        in_maps.append({
            "xT": xTh[g],
            "wall": np.ascontiguousarray(
                wall.reshape(P, 4 * NCH * DL)).astype(bf),
        })
    return in_maps


def assemble_output(results):
    outs = []
    for g in range(B):
        outT = np.concatenate(
            [np.asarray(results[GW * g + i]["out"], np.float32)
             for i in range(GW)], axis=0)  # [C, TL]
        outs.append(outT.T)
    return np.ascontiguousarray(np.stack(outs))  # [B, T, C]


def kernel(x, Wq, Wk, Wv, Wo):
    nc = _get_graph()
    in_maps = prepare_in_maps(x, Wq, Wk, Wv, Wo)
    res = run_bass_kernel_spmd(nc, in_maps, core_ids=list(range(N_CORES)))
    return assemble_output(res.results)
